# revision 1
# baseline (speedup 1.0000x reference)
"""DGT block (dynamic graph transformer) Bass kernel for Trainium2.

Sharding: 8 cores = 4 batches x 2 query-halves. Each core's inputs are
column-rotated so its own 2048 queries are always columns 0:NQ; all
per-core index bookkeeping is in rotated space (self-consistent).

Key structure per core:
  - kNN scores 2^17*(f_n . f_j - 0.5||f_j||^2) via bf16 hi/lo
    split-precision matmuls (hi*hi + one stacked 128-row cross-term
    matmul), with the norm row and the rounding constant C = 1.5*2^35
    entering as a final 3-row bf16 matmul so the fp32 PSUM accumulate
    rounds the score to a multiple of 4096.
  - packed top-16: a DVE scalar_tensor_tensor reads PSUM, subtracts C
    and adds the column index, so every score carries its index in the
    low 12 bits. Chunked max8 + match_replace then yields the top-16
    (value, index) pairs with no MaxIndex scans; indices are recovered
    exactly with a round-to-integer trick (all in fp32 integers).
  - gather table rows [gk | v | gp] f16 in DRAM; 4x 512-index
    dma_gather(transpose=True) per query tile (transpose mode is
    limited to 512 indices per call on hardware).
  - attention MLPs as bf16 matmuls with PSUM accumulation through two
    2-bank PSUM half-rings; per-query broadcast terms enter via an
    expander matmul and gathered terms via a (-I) matmul.
  - softmax over the 16 neighbors via fp16 2x-mode halving-tree adds,
    with Pool taking ww/uu off the DVE in steady state.
  - emission is software-pipelined (stages A/B1/Bg/C1/C2/D offset per
    cycle) because each engine executes its instruction stream in order.
"""

import numpy as np
import ml_dtypes

B, N, K, DP, DM, EPS = 4, 4096, 16, 64, 128, 1e-5
NQ = N // 2            # queries per core
TQ = 128               # queries per tile
NT = NQ // TQ          # tiles per core (16)
PAIR = TQ * K          # pairs per tile (2048)
CH = 512               # top-k scan chunk size
NCH = N // CH          # 8 chunks
ROW = 3 * DM           # gather-table row elems (f16): [gk(128) | v(128) | gp(128)]

SC_SCALE = 131072.0          # 2^17: score quant step = 4096/2^17 = 1/32
C1 = 51539607552.0           # 1.5 * 2^35: rounds (s*2^17) to multiples of 4096
C2 = 12582912.0              # 1.5 * 2^23: rounds to integer
INV4096 = 1.0 / 4096.0

_CACHE = {}

bf16 = ml_dtypes.bfloat16


def _fold_bn(p):
    g, be, m, v = p.astype(np.float64)
    s = g / np.sqrt(v + EPS)
    return (s).astype(np.float32), (be - m * s).astype(np.float32)


def _build_bass():
    import concourse.bass as bass
    import concourse.mybir as mybir
    import concourse.bacc as bacc
    from concourse.tile import TileContext

    dt = mybir.dt
    AF = mybir.ActivationFunctionType
    ALU = mybir.AluOpType
    AX = mybir.AxisListType

    nc = bacc.Bacc("TRN2", target_bir_lowering=False, debug=False, num_devices=8)

    # ---- I/O ----
    def inp(name, shape, dtype):
        return nc.dram_tensor(name, list(shape), dtype, kind="ExternalInput").ap()

    feats_f32 = inp("feats_f32", (DP, N), dt.float32)
    flh8_d = inp("FLH8", (2 * DP, N), dt.bfloat16)
    fhi8_d = inp("FHI8", (DP, N), dt.bfloat16)
    c1row_d = inp("C1ROW", (1, N), dt.bfloat16)
    fhl9_d = inp("FHL9", (2 * DP, NQ), dt.bfloat16)
    feats_bf = inp("feats_bf", (DP, N), dt.bfloat16)
    pos_bf = inp("pos_bf", (3, N), dt.bfloat16)
    w1t_d = inp("W1fT", (DP, DM), dt.bfloat16)
    wkvt_d = inp("WgkvT", (DM, 2 * DM), dt.bfloat16)
    wqt_d = inp("Wg1qT", (DM, DM), dt.bfloat16)
    wd1t_d = inp("Wd1fT", (3, DM), dt.bfloat16)
    wd2t_d = inp("Wd2fT", (DM, DM), dt.bfloat16)
    wg1t_d = inp("Wg1fT", (DM, DM), dt.bfloat16)
    wg2t_d = inp("Wg2fT", (DM, DM), dt.bfloat16)
    w2t_d = inp("W2fT", (DM, DP), dt.bfloat16)
    e_d = inp("E", (TQ, PAIR), dt.bfloat16)
    negi_d = inp("negI", (DM, DM), dt.float16)
    ident_d = inp("ident", (DM, DM), dt.float32)
    b1_d = inp("b1", (DM, 1), dt.float32)
    bd1_d = inp("bd1", (DM, 1), dt.float32)
    bd2_d = inp("bd2", (DM, 1), dt.float32)
    bg1_d = inp("bg1", (DM, 1), dt.float32)
    bg2_d = inp("bg2", (DM, 1), dt.float32)
    b2_d = inp("b2", (DP, 1), dt.float32)

    out_d = nc.dram_tensor("out", [DP, NQ], dt.float32, kind="ExternalOutput").ap()

    f32, f32r, f16, bft, i16 = dt.float32, dt.float32r, dt.float16, dt.bfloat16, dt.int16

    def r(ap):  # float32r view for fast fp32 matmuls
        return ap.bitcast(f32r)

    with TileContext(nc) as tc:
        with (
            tc.tile_pool(name="const", bufs=1) as cpool,
            tc.tile_pool(name="persist", bufs=1) as ppool,
            tc.tile_pool(name="dram", bufs=1, space="DRAM") as dpool,
        ):
            # persistent working tensors; critical input DMAs are emitted
            # first so phase A compute can start as soon as possible.
            fT = ppool.tile([DP, N], f32)
            nc.sync.dma_start(out=fT[:], in_=feats_f32)
            flh8 = ppool.tile([2 * DP, N], bft)
            nc.sync.dma_start(out=flh8[:], in_=flh8_d)
            fhi8 = ppool.tile([DP, N], bft)
            nc.sync.dma_start(out=fhi8[:], in_=fhi8_d)
            fhl9 = ppool.tile([2 * DP, NQ], bft)
            nc.sync.dma_start(out=fhl9[:], in_=fhl9_d)
            norms3 = ppool.tile([3, N], bft)
            gqp = ppool.tile([TQ, NT * 2 * DM], bft)  # [gq_c | gp_c] per chunk
            res_all = ppool.tile([DM, NQ], bft)
            table = dpool.tile([N, ROW], f16)

            # constants (weights needed by phase A first)
            w1t = cpool.tile_from(w1t_d)
            wkvt = cpool.tile_from(wkvt_d)
            wqt = cpool.tile_from(wqt_d)
            wd1t = cpool.tile_from(wd1t_d)
            wd2t = cpool.tile_from(wd2t_d)
            wg1t = cpool.tile_from(wg1t_d)
            wg2t = cpool.tile_from(wg2t_d)
            w2t = cpool.tile_from(w2t_d)
            emat = cpool.tile_from(e_d)
            negi = cpool.tile_from(negi_d)
            ident = cpool.tile_from(ident_d)
            b1 = cpool.tile_from(b1_d)
            bd1 = cpool.tile_from(bd1_d)
            bd2 = cpool.tile_from(bd2_d)
            bg1 = cpool.tile_from(bg1_d)
            bg2 = cpool.tile_from(bg2_d)
            b2 = cpool.tile_from(b2_d)
            iota = cpool.tile([TQ, N], i16)
            nc.gpsimd.iota(iota[:], pattern=[[1, N]], base=0,
                           channel_multiplier=0)
            ones3 = cpool.tile([3, TQ], bft)
            nc.vector.memset(ones3[:], 1.0)
            nc.sync.dma_start(out=norms3[2:3, :], in_=c1row_d)
            ones64 = cpool.tile([DP, 1], f32)
            nc.vector.memset(ones64[:], 1.0)

            # ---- Phase A prologue: feature norms + x = lrelu(bn(W1 f)) ----
            with (
                tc.tile_pool(name="score", bufs=2) as spool,
                tc.tile_pool(name="topk", bufs=2) as kpool,
                tc.tile_pool(name="ps_s", bufs=3, space="PSUM") as pss,
                tc.tile_pool(name="ps_t", bufs=1, space="PSUM") as pst,
            ):
                xpool_cm = tc.tile_pool(name="xpool", bufs=1)
                xpool = xpool_cm.__enter__()
                apool_cm = tc.tile_pool(name="setupA", bufs=2)
                apool = apool_cm.__enter__()
                fbt = xpool.tile([DP, N], bft)
                nc.sync.dma_start(out=fbt[:], in_=feats_bf)
                post = xpool.tile([3, N], bft)
                nc.sync.dma_start(out=post[:], in_=pos_bf)
                xfull = xpool.tile([DM, N], bft)

                with tc.tile_pool(name="ps_pre", bufs=2, space="PSUM") as pspre:
                    for s in range(8):
                        ff = apool.tile([DP, 512], f32, tag="ff")
                        nc.gpsimd.tensor_mul(ff[:], fT[:, bass.ts(s, 512)],
                                             fT[:, bass.ts(s, 512)])
                        ps = pspre.tile([1, 512], f32, tag="pssq")
                        nc.tensor.matmul(ps[:], ones64[:], ff[:],
                                         start=True, stop=True)
                        nf = apool.tile([1, 512], f32, tag="nf")
                        nc.scalar.activation(nf[:], ps[:], AF.Copy, bias=0.0,
                                             scale=-0.5 * SC_SCALE)
                        # hi/lo bf16 split of the scaled norm row; rows 1..2 of
                        # norms3 are off-base partitions, reachable only by DMA
                        nc.scalar.copy(norms3[0:1, bass.ts(s, 512)], nf[:])
                        nl = apool.tile([1, 512], bft, tag="nl", bufs=4)
                        nc.vector.tensor_sub(nl[:], nf[:],
                                             norms3[0:1, bass.ts(s, 512)])
                        nc.sync.dma_start(out=norms3[1:2, bass.ts(s, 512)],
                                          in_=nl[:])
                        psx = pspre.tile([DM, 512], f32, tag="psx")
                        nc.tensor.matmul(psx[:], w1t[:], fbt[:, bass.ts(s, 512)],
                                         start=True, stop=True)
                        nc.scalar.activation(xfull[:, bass.ts(s, 512)], psx[:],
                                             AF.Prelu, bias=b1[:], scale=1.0,
                                             alpha=0.2)

                # ---- staged per-tile pipeline ----
                st = [dict() for _ in range(NT)]

                def stage_a(t):
                    # scores -> C1-biased quantized via matmul; DVE stt packs
                    # the column index straight out of PSUM; chunked max8.
                    cand = kpool.tile([TQ, NCH * 8], f32, tag="cand", bufs=3)
                    st[t]["cand"] = cand
                    for hf in range(2):
                        sc = spool.tile([TQ, N // 2], f32, tag="sch")
                        for g in range(2):
                            pses = []
                            for s4 in range(2 * g, 2 * g + 2):
                                pses.append(pss.tile([TQ, 512], f32, tag="pssc", name="pssc"))
                            for s4 in range(2 * g, 2 * g + 2):
                                nc.tensor.matmul(pses[s4 - 2 * g][:],
                                                 fhl9[0:DP, bass.ts(t, TQ)],
                                                 fhi8[:, bass.ts(hf * 4 + s4, 512)],
                                                 start=True, stop=False)
                            for s4 in range(2 * g, 2 * g + 2):
                                nc.tensor.matmul(pses[s4 - 2 * g][:],
                                                 fhl9[:, bass.ts(t, TQ)],
                                                 flh8[:, bass.ts(hf * 4 + s4, 512)],
                                                 start=False, stop=False)
                            for s4 in range(2 * g, 2 * g + 2):
                                nc.tensor.matmul(pses[s4 - 2 * g][:], ones3[:],
                                                 norms3[:, bass.ts(hf * 4 + s4, 512)],
                                                 start=False, stop=True)
                            for s4 in range(2 * g, 2 * g + 2):
                                nc.vector.scalar_tensor_tensor(
                                    out=sc[:, bass.ts(s4, 512)],
                                    in0=pses[s4 - 2 * g][:],
                                    scalar=-C1,
                                    in1=iota[:, bass.ts(hf * 4 + s4, 512)],
                                    op0=ALU.add, op1=ALU.add)
                        for c in range(NCH // 2):
                            nc.vector.max(out=cand[:, bass.ts(hf * 4 + c, 8)],
                                          in_=sc[:, bass.ts(c, CH)])

                def stage_b1(t):
                    # top-16 merge over candidates + index extraction
                    cand = st[t].pop("cand")
                    pk = kpool.tile([TQ, 16], f32, tag="pk")
                    nc.vector.max(out=pk[:, 0:8], in_=cand[:])
                    repl = kpool.tile([TQ, NCH * 8], f32, tag="repl")
                    nc.vector.match_replace(out=repl[:], in_to_replace=pk[:, 0:8],
                                            in_values=cand[:], imm_value=-1e30)
                    nc.vector.max(out=pk[:, 8:16], in_=repl[:])

                    # j = pk mod 4096 (exact fp32)
                    uq = kpool.tile([TQ, 16], f32, tag="uq")
                    nc.vector.tensor_scalar_mul(uq[:], pk[:], INV4096)
                    vq = kpool.tile([TQ, 16], f32, tag="vq")
                    nc.vector.tensor_scalar(out=vq[:], in0=uq[:], scalar1=C2,
                                            scalar2=-C2, op0=ALU.add, op1=ALU.add)
                    wq = kpool.tile([TQ, 16], f32, tag="wq")
                    nc.vector.scalar_tensor_tensor(out=wq[:], in0=vq[:],
                                                   scalar=-4096.0, in1=pk[:],
                                                   op0=ALU.mult, op1=ALU.add)
                    mq = kpool.tile([TQ, 16], f32, tag="mq")
                    nc.vector.tensor_scalar(out=mq[:], in0=wq[:], scalar1=0.0,
                                            scalar2=4096.0, op0=ALU.is_lt,
                                            op1=ALU.mult)
                    idxf = kpool.tile([TQ, DM], f32, tag="idxf")
                    nc.vector.tensor_add(idxf[:, 0:16], wq[:], mq[:])
                    nc.scalar.copy(idxf[:, 16:32], idxf[:, 0:16])
                    nc.scalar.copy(idxf[:, 32:64], idxf[:, 0:32])
                    nc.scalar.copy(idxf[:, 64:128], idxf[:, 0:64])
                    pt = pst.tile([DM, TQ], f32, tag="pst")
                    nc.tensor.transpose(pt[:], idxf[:], ident[:])
                    idx16 = kpool.tile([TQ, TQ], i16, tag="idx16", bufs=3)
                    st[t]["idx16"] = idx16
                    nc.scalar.copy(idx16[:], pt[:])

                def stage_bg(t):
                    # transpose-mode dma_gather is limited to 512 idxs/call
                    idx16 = st[t].pop("idx16")
                    gkv = gpool.tile([DM, 4, 3, 512], f16, tag="gkv", bufs=4)
                    st[t]["gkv"] = gkv
                    for gh in range(4):
                        nc.gpsimd.dma_gather(
                            out_ap=gkv[:, gh, :, :], in_ap=table[:],
                            idxs_ap=idx16[:, bass.ts(gh, 32)],
                            num_idxs=512, num_idxs_reg=512, elem_size=ROW,
                            transpose=True)

                HP = PAIR // 2

                def stage_c1(t):
                    # pe MLP in two independent 1024-pair halves, one through
                    # each 2-bank PSUM half-ring, so the serial chain halves
                    # and consecutive tiles' halves overlap.
                    gp_l = gqp[:, t * 2 * DM + DM: (t + 1) * 2 * DM]
                    gkv = st[t]["gkv"]
                    h1 = prpool.tile([DM, PAIR], bft, tag="h1", bufs=2)
                    zph = [None, None]
                    for hx, zpool in ((0, pspL), (1, pspR)):
                        zp = zpool.tile([DM, HP], f32, tag="zh")
                        zph[hx] = zp
                        for hh in (2 * hx, 2 * hx + 1):
                            sl2 = bass.ts(hh - 2 * hx, 512)
                            nc.tensor.matmul(zp[:, sl2], gp_l,
                                             emat[:, bass.ts(hh, 512)],
                                             start=True, stop=False)
                            nc.tensor.matmul(zp[:, sl2], negi[:],
                                             gkv[:, hh, 2, :],
                                             start=False, stop=True)
                    for hx in (0, 1):
                        nc.scalar.activation(h1[:, bass.ts(hx, HP)], zph[hx][:],
                                             AF.Prelu, bias=bd1[:],
                                             scale=1.0, alpha=0.2)
                    pe = prpool.tile([DM, PAIR], bft, tag="pe", bufs=2)
                    st[t]["pe"] = pe
                    for hx, zpool in ((0, pspL), (1, pspR)):
                        zp = zpool.tile([DM, HP], f32, tag="zh")
                        zph[hx] = zp
                        for hh in (2 * hx, 2 * hx + 1):
                            sl = bass.ts(hh, 512)
                            sl2 = bass.ts(hh - 2 * hx, 512)
                            nc.tensor.matmul(zp[:, sl2], wd2t[:], h1[:, sl],
                                             start=True, stop=True)
                    for hx in (0, 1):
                        nc.scalar.activation(pe[:, bass.ts(hx, HP)], zph[hx][:],
                                             AF.Prelu, bias=bd2[:],
                                             scale=1.0, alpha=0.2)

                def stage_c2(t):
                    # attention MLP, same half-ring structure
                    gq_l = gqp[:, t * 2 * DM: t * 2 * DM + DM]
                    gkv = st[t]["gkv"]
                    pe = st[t]["pe"]
                    a1 = prpool.tile([DM, PAIR], bft, tag="a1", bufs=2)
                    zph = [None, None]
                    for hx, zpool in ((0, pspL), (1, pspR)):
                        zp = zpool.tile([DM, HP], f32, tag="zh")
                        zph[hx] = zp
                        for hh in (2 * hx, 2 * hx + 1):
                            sl = bass.ts(hh, 512)
                            sl2 = bass.ts(hh - 2 * hx, 512)
                            nc.tensor.matmul(zp[:, sl2], gq_l,
                                             emat[:, bass.ts(hh, 512)],
                                             start=True, stop=False)
                            nc.tensor.matmul(zp[:, sl2], negi[:],
                                             gkv[:, hh, 0, :],
                                             start=False, stop=False)
                            nc.tensor.matmul(zp[:, sl2], wg1t[:],
                                             pe[:, sl],
                                             start=False, stop=True)
                    for hx in (0, 1):
                        nc.scalar.activation(a1[:, bass.ts(hx, HP)], zph[hx][:],
                                             AF.Prelu, bias=bg1[:],
                                             scale=1.0, alpha=0.2)
                    a2 = prpool.tile([DM, PAIR], bft, tag="a2", bufs=2)
                    ee = prpool.tile([DM, PAIR], f16, tag="ee", bufs=3)
                    st[t]["ee"] = ee
                    for hx, zpool in ((0, pspL), (1, pspR)):
                        zp = zpool.tile([DM, HP], f32, tag="zh")
                        zph[hx] = zp
                        for hh in (2 * hx, 2 * hx + 1):
                            sl = bass.ts(hh, 512)
                            sl2 = bass.ts(hh - 2 * hx, 512)
                            nc.tensor.matmul(zp[:, sl2], wg2t[:], a1[:, sl],
                                             start=True, stop=True)
                    for hx in (0, 1):
                        nc.scalar.activation(a2[:, bass.ts(hx, HP)], zph[hx][:],
                                             AF.Prelu, bias=bg2[:],
                                             scale=1.0, alpha=0.2)
                        nc.scalar.activation(ee[:, bass.ts(hx, HP)],
                                             a2[:, bass.ts(hx, HP)], AF.Exp,
                                             bias=0.0, scale=1.0 / 64.0)
                    ww = prpool.tile([DM, PAIR], f16, tag="ww", bufs=3)
                    st[t]["ww"] = ww
                    weng = nc.vector if t >= NT - 3 else nc.gpsimd
                    weng.tensor_add(ww[:].rearrange('p (g x) -> p g x', g=4),
                                    gkv[:, :, 1, :],
                                    pe[:].rearrange('p (g x) -> p g x', g=4))

                def stage_d(t):
                    # softmax-normalized weighted sum via halving trees;
                    # wide first levels on Pool, narrow tails on DVE.
                    ee, ww = st[t].pop("ee"), st[t].pop("ww")
                    st[t].pop("gkv"), st[t].pop("pe")
                    uu = prpool.tile([DM, PAIR], f16, tag="uu", bufs=1)
                    ueng = nc.vector if t >= NT - 3 else nc.gpsimd
                    ueng.tensor_mul(uu[:], ee[:], ww[:])

                    def tree_sum(src, pfx):
                        s3 = src[:].rearrange("p (q k) -> p q k", k=K)
                        l1 = tpool.tile([DM, TQ, 8], f16, tag=pfx + "l1")
                        nc.vector.tensor_add(l1[:], s3[:, :, 0:8], s3[:, :, 8:16])
                        l2 = tpool.tile([DM, TQ, 4], f16, tag=pfx + "l2")
                        nc.vector.tensor_add(l2[:], l1[:, :, 0:4], l1[:, :, 4:8])
                        l3 = tpool.tile([DM, TQ, 2], f16, tag=pfx + "l3")
                        nc.vector.tensor_add(l3[:], l2[:, :, 0:2], l2[:, :, 2:4])
                        l4 = tpool.tile([DM, TQ, 1], f32, tag=pfx + "l4")
                        nc.vector.tensor_add(l4[:], l3[:, :, 0:1], l3[:, :, 1:2])
                        return l4

                    ssum = tree_sum(ee, "se")
                    rrec = tpool.tile([DM, TQ], f32, tag="rrec")
                    nc.vector.reciprocal(rrec[:], ssum[:].rearrange("p q k -> p (q k)"))
                    ru = tree_sum(uu, "su")
                    nc.vector.tensor_mul(res_all[:, bass.ts(t, TQ)],
                                         ru[:].rearrange("p q k -> p (q k)"),
                                         rrec[:])

                def emit_table_chunk(c):
                    pt5 = psmid.tile([TQ, 512], f32, tag="pstab")
                    nc.tensor.matmul(pt5[:, 0:2 * DM], xfull[:, bass.ts(c, TQ)],
                                     wkvt[:], start=True, stop=True)
                    nc.tensor.matmul(pt5[:, 2 * DM:ROW], post[:, bass.ts(c, TQ)],
                                     wd1t[:], start=True, stop=True)
                    stg = apool.tile([TQ, ROW], f16, tag="stg", bufs=8)
                    nc.scalar.activation(stg[:], pt5[:, 0:ROW], AF.Copy)
                    nc.sync.dma_start(out=table[bass.ts(c, TQ), :], in_=stg[:])

                def emit_gqp_chunk(c):
                    pq = psmid.tile([TQ, 2 * DM], f32, tag="psgq")
                    nc.tensor.matmul(pq[:, 0:DM], xfull[:, bass.ts(c, TQ)],
                                     wqt[:], start=True, stop=True)
                    nc.tensor.matmul(pq[:, DM:2 * DM], post[:, bass.ts(c, TQ)],
                                     wd1t[:], start=True, stop=True)
                    nc.scalar.activation(gqp[:, bass.ts(c, 2 * DM)], pq[:],
                                         AF.Copy)

                with tc.tile_pool(name="ps_mid", bufs=2, space="PSUM") as psmid:
                    for cyc in range(8):
                        for c in range(4 * cyc, 4 * cyc + 4):
                            emit_table_chunk(c)
                        for c in range(2 * cyc, 2 * cyc + 2):
                            emit_gqp_chunk(c)
                        if cyc >= 6:
                            stage_a(cyc - 6)
                        if cyc >= 7:
                            stage_b1(cyc - 7)

                apool_cm.__exit__(None, None, None)
                xpool_cm.__exit__(None, None, None)
                with (
                    tc.tile_pool(name="gath", bufs=1) as gpool,
                    tc.tile_pool(name="pair", bufs=2) as prpool,
                    tc.tile_pool(name="tree", bufs=1) as tpool,
                    tc.tile_pool(name="ps_pairL", bufs=1, space="PSUM") as pspL,
                    tc.tile_pool(name="ps_pairR", bufs=1, space="PSUM") as pspR,
                ):
                    for cyc in range(8, 29):
                        if 0 <= cyc - 6 < NT:
                            stage_a(cyc - 6)
                        if 0 <= cyc - 7 < NT:
                            stage_b1(cyc - 7)
                        if 0 <= cyc - 8 < NT:
                            stage_bg(cyc - 8)
                        if 0 <= cyc - 9 < NT:
                            stage_c1(cyc - 9)
                        if 0 <= cyc - 10 < NT:
                            stage_c2(cyc - 10)
                        if 0 <= cyc - 12 < NT:
                            stage_d(cyc - 12)

            # ---------------- Phase C: output ----------------
            with (
                tc.tile_pool(name="outp", bufs=2) as opool,
                tc.tile_pool(name="ps_o", bufs=2, space="PSUM") as pso,
            ):
                o1 = opool.tile([DP, NQ], f32, tag="o1")
                for s in range(4):
                    ps = pso.tile([DP, 512], f32, tag="pso")
                    nc.tensor.matmul(ps[:], w2t[:], res_all[:, bass.ts(s, 512)],
                                     start=True, stop=True)
                    nc.scalar.activation(o1[:, bass.ts(s, 512)], ps[:], AF.Prelu,
                                         bias=b2[:], scale=1.0, alpha=0.2)
                o2 = opool.tile([DP, NQ], f32, tag="o2")
                nc.vector.tensor_add(o2[:], o1[:], fT[0:DP, 0:NQ])
                nc.sync.dma_start(out=out_d, in_=o2[:])

    nc.compile()
    return nc


def _host_prep(inputs):
    """Fold BN into weights, build per-core input maps (column-rotated)."""
    s1, b1 = _fold_bn(np.asarray(inputs["bn1"]))
    sd1, bd1 = _fold_bn(np.asarray(inputs["bnd1"]))
    sd2, bd2 = _fold_bn(np.asarray(inputs["bnd2"]))
    sg1, bg1 = _fold_bn(np.asarray(inputs["bng1"]))
    sg2, bg2 = _fold_bn(np.asarray(inputs["bng2"]))
    s2, b2 = _fold_bn(np.asarray(inputs["bn2"]))
    W1f = np.asarray(inputs["W1"]) * s1[:, None]
    Wd1f = np.asarray(inputs["Wd1"]) * sd1[:, None]
    Wd2f = np.asarray(inputs["Wd2"]) * sd2[:, None]
    Wg1f = np.asarray(inputs["Wg1"]) * sg1[:, None]
    Wg2f = np.asarray(inputs["Wg2"]) * sg2[:, None]
    W2f = np.asarray(inputs["W2"]) * s2[:, None]
    Wg1k = (Wg1f @ np.asarray(inputs["Wk"])).astype(np.float32)
    Wg1q = (Wg1f @ np.asarray(inputs["Wq"])).astype(np.float32)
    Wv = np.asarray(inputs["Wv"], np.float32)

    E = np.zeros((TQ, PAIR), np.float32)
    for q in range(TQ):
        E[q, q * K:(q + 1) * K] = 1.0

    com = {
        "W1fT": np.ascontiguousarray(W1f.T, dtype=bf16),
        "WgkvT": np.ascontiguousarray(
            np.concatenate([Wg1k.T, Wv.T], axis=1), dtype=bf16),
        "Wg1qT": np.ascontiguousarray(Wg1q.T, dtype=bf16),
        "Wd1fT": np.ascontiguousarray(Wd1f.T, dtype=bf16),
        "Wd2fT": np.ascontiguousarray(Wd2f.T, dtype=bf16),
        "Wg1fT": np.ascontiguousarray(Wg1f.T, dtype=bf16),
        "Wg2fT": np.ascontiguousarray(Wg2f.T, dtype=bf16),
        "W2fT": np.ascontiguousarray(W2f.T, dtype=bf16),
        "E": E.astype(bf16),
        "negI": (-np.eye(DM)).astype(np.float16),
        "ident": np.eye(DM, dtype=np.float32),
        "b1": b1.reshape(DM, 1),
        "bd1": bd1.reshape(DM, 1),
        "bd2": bd2.reshape(DM, 1),
        "bg1": bg1.reshape(DM, 1),
        "bg2": bg2.reshape(DM, 1),
        "b2": b2.reshape(DP, 1),
    }

    feats = np.asarray(inputs["feats"], np.float32)
    pos = np.asarray(inputs["pos"], np.float32)
    in_maps = []
    for c in range(8):
        b, h = c // 2, c % 2
        n0 = h * NQ
        # rotate columns so own queries are always cols 0:NQ
        fb = np.concatenate([feats[b][:, n0:], feats[b][:, :n0]], axis=1)
        pb = np.concatenate([pos[b][:, n0:], pos[b][:, :n0]], axis=1)
        m = dict(com)
        m["feats_f32"] = np.ascontiguousarray(fb)
        m["feats_bf"] = np.ascontiguousarray(fb, dtype=bf16)
        m["pos_bf"] = np.ascontiguousarray(pb, dtype=bf16)
        fhi = fb.astype(bf16).astype(np.float32)
        flo = (fb - fhi).astype(bf16).astype(np.float32)
        m["FLH8"] = np.ascontiguousarray(
            np.concatenate([flo * 256.0, fhi * 256.0], axis=0), dtype=bf16)
        m["FHI8"] = np.ascontiguousarray(fhi * 256.0, dtype=bf16)
        m["C1ROW"] = np.full((1, N), C1, dtype=bf16)
        m["FHL9"] = np.ascontiguousarray(
            np.concatenate([fhi[:, 0:NQ] * 512.0, flo[:, 0:NQ] * 512.0],
                           axis=0), dtype=bf16)
        in_maps.append(m)
    return in_maps


def kernel(**inputs):
    from concourse.bass_utils import run_bass_kernel_spmd

    if "nc" not in _CACHE:
        _CACHE["nc"] = _build_bass()
    nc = _CACHE["nc"]
    in_maps = _host_prep(inputs)
    r = run_bass_kernel_spmd(nc, in_maps, core_ids=list(range(8)),
                             **_CACHE.get("run_kwargs", {}))
    _CACHE["last_result"] = r
    out = np.empty((B, DP, N), np.float32)
    for c in range(8):
        b, h = c // 2, c % 2
        out[b][:, h * NQ:(h + 1) * NQ] = r.results[c]["out"]
    return out

